# revision 7
# baseline (speedup 1.0000x reference)
"""Pointer-network decode kernel for 8 Trainium2 NeuronCores.

Data-parallel over batch: B=64 -> 8 batches per core. Each core runs the
full T-step attention/GRU decode for its batch slice; the host shards
inputs (with layout pre-transposes/casts) and concatenates outputs.

Device-side algorithm per core (B_loc=8, N=2048, D=H=P=256):
  keep sequence resident in SBUF in BOTH layouts (bf16):
    seqT [d-part, (b, dh, n)]  -- moving operand of the projection matmul
    seqO [n-part, (b, c, d)]   -- stationary operand of the context matmul
  per step:
    hW   = W_h^T h            (PE, output transposed [p, b])
    proj = W_seq^T seqT       (PE, bf16 -> psum fp32, 1024-wide, t=0 only;
                               cached bf16 in SBUF for t>0)
    tanh = Tanh(proj + hW_b)  (ACT, per-partition bias, psum -> sbuf bf16)
    score= v8_b^T tanh        (PE; v padded into column b accumulates all
                               batches into one [8, 4, 512] psum; the mask
                               bias rows are added by one identity-matmul
                               per chunk)
    softmax over n: exp straight off psum (scores are bounded by ||v||_1,
                    so no max subtraction), accum_out gives the row sums
    w    = e * (1/sum)        (DVE, also the DMA'd output)
  and for every step except the last (h_T is never observed):
    wT   = transpose(w)       (PE transpose, 128-chunks)
    vecT = seqO^T wT_b        (PE, accumulated over n-chunks, [1, d] rows)
    GRU with weights as the MOVING operand: gates land as [8, 768] rows
      (vecT/hT [128, 8] stationary, W^T [128, 768] bf16 moving), gate math
      on 8-partition rows via DVE/ACT, h' transposed back to [d-part, b].
"""

import numpy as np
import ml_dtypes

B, N, D, H, P = 64, 2048, 256, 256, 256
NCORES = 8
BL = B // NCORES          # batch per core
NEG_INF = -1e9

_CACHE = {}


def _build(T: int, split: bool = True):
    """Build the Bass program (one core's SPMD program) for T decode steps."""
    from contextlib import ExitStack
    import concourse.bass as bass
    import concourse.tile as tile
    from concourse import mybir, masks

    f32 = mybir.dt.float32
    bf16 = mybir.dt.bfloat16
    AF = mybir.ActivationFunctionType
    ALU = mybir.AluOpType
    AX = mybir.AxisListType

    nc = bass.Bass()

    # ---- DRAM I/O (per-core shapes) ----
    d_seqT = nc.dram_tensor("seqT", [128, BL, 2, N], bf16, kind="ExternalInput")
    d_seqO = nc.dram_tensor("seqO", [128, BL, N // 128, D], bf16, kind="ExternalInput")
    d_wseq = nc.dram_tensor("wseq", [128, 2, P], bf16, kind="ExternalInput")
    d_v8 = nc.dram_tensor("v8", [128, 2, BL, BL], bf16, kind="ExternalInput")
    d_h0T = nc.dram_tensor("h0T", [128, 2, BL], bf16, kind="ExternalInput")
    d_h0r = nc.dram_tensor("h0r", [BL, H], f32, kind="ExternalInput")
    d_wh = nc.dram_tensor("wh", [128, 2, P], bf16, kind="ExternalInput")
    d_wihT = nc.dram_tensor("wihT", [128, 2, 3 * H], bf16, kind="ExternalInput")
    d_whhT = nc.dram_tensor("whhT", [128, 2, 3 * H], bf16, kind="ExternalInput")
    d_bih = nc.dram_tensor("bih", [1, 3 * H], f32, kind="ExternalInput")
    d_bhh = nc.dram_tensor("bhh", [1, 3 * H], f32, kind="ExternalInput")
    d_mb = nc.dram_tensor("mb", [BL, N], f32, kind="ExternalInput")
    d_wout = nc.dram_tensor("wout", [BL, T, N], f32, kind="ExternalOutput")

    NC4 = N // 512    # 4 score chunks of 512
    NCK = N // 1024   # 2 proj/tanh chunks of 1024
    NC128 = N // 128  # 16 chunks of 128 for transpose/vec

    with tile.TileContext(nc) as tc, ExitStack() as ctx:
        cpool = ctx.enter_context(tc.tile_pool(name="consts", bufs=1))
        seqp = ctx.enter_context(tc.tile_pool(name="seq", bufs=17 if T > 1 else 9))
        tanhp = ctx.enter_context(tc.tile_pool(name="tanh", bufs=4))
        rowp = ctx.enter_context(tc.tile_pool(name="rows", bufs=2))
        hp = ctx.enter_context(tc.tile_pool(name="hstate", bufs=2))
        # PSUM: tag "pr" 2 slots x [128,1024]f32 (2 banks each) +
        #       tag "sc" 1 slot  x [8,2,1024]f32 (4 banks) = 8 banks exactly.
        pmm = ctx.enter_context(tc.tile_pool(name="pmm", bufs=2, space="PSUM"))
        psc = ctx.enter_context(tc.tile_pool(name="psc", bufs=1, space="PSUM"))

        # ---- constants / weights into SBUF ----
        ident = cpool.tile([128, 128], f32, tag="ident")
        masks.make_identity(nc, ident[:])
        ones8 = cpool.tile([1, BL], f32, tag="ones8")
        nc.gpsimd.memset(ones8[:], 1.0)

        wh_sb = cpool.tile([128, 2, P], bf16, tag="wh")
        nc.sync.dma_start(wh_sb[:], d_wh[:])
        hT_bf = hp.tile([128, 2, BL], bf16, tag="hT")
        nc.sync.dma_start(hT_bf[:], d_h0T[:])
        h_rows = hp.tile([BL, H], f32, tag="h_rows")
        nc.sync.dma_start(h_rows[:], d_h0r[:])
        wseq_sb = cpool.tile([128, 2, P], bf16, tag="wseq")
        nc.sync.dma_start(wseq_sb[:], d_wseq[:])
        v8_sb = cpool.tile([128, 2, BL, BL], bf16, tag="v8")
        nc.sync.dma_start(v8_sb[:], d_v8[:])
        mb_sb = cpool.tile([BL, N], f32, tag="mb")
        nc.sync.dma_start(mb_sb[:], d_mb[:])

        # ---- sequence loads: seqT first (step-0 critical path), then
        # seqO (only needed at context time). All share one 8KB/partition
        # tile tag so freed seqT slots are recycled for proj cache / seqO.
        seqT_b = []
        for b in range(BL):
            tl = seqp.tile([128, 2, N], bf16, tag="blk8", name=f"seqT{b}")
            nc.sync.dma_start(tl[:], d_seqT[:, b, :, :])
            seqT_b.append(tl)

        # remaining constants are not needed until GRU time
        wihT_sb = cpool.tile([128, 2, 3 * H], bf16, tag="wihT")
        nc.sync.dma_start(wihT_sb[:], d_wihT[:])
        whhT_sb = cpool.tile([128, 2, 3 * H], bf16, tag="whhT")
        nc.sync.dma_start(whhT_sb[:], d_whhT[:])
        bih_sb = cpool.tile([1, 3 * H], f32, tag="bih")
        nc.sync.dma_start(bih_sb[:], d_bih[:])
        bhh_sb = cpool.tile([1, 3 * H], f32, tag="bhh")
        nc.sync.dma_start(bhh_sb[:], d_bhh[:])

        seqO_b = []
        if T > 1:
            for b in range(BL):
                tl = seqp.tile([128, N // 128, D], bf16, tag="blk8",
                               name=f"seqO{b}")
                nc.sync.dma_start(tl[:], d_seqO[:, b, :, :])
                seqO_b.append(tl)
        proj_b = [None] * BL   # filled at t=0, reused t>=1

        for t in range(T):
            last = (t == T - 1)

            # ---- hW^T = W_h^T h  -> [p(2x128), b] ----
            p_hw = pmm.tile([128, 2, BL], f32, tag="pr", name=f"p_hw{t}")
            for mh in range(2):
                for kh in range(2):
                    nc.tensor.matmul(
                        p_hw[:, mh, :],
                        wh_sb[:, kh, mh * 128:(mh + 1) * 128],
                        hT_bf[:, kh, :],
                        start=(kh == 0), stop=(kh == 1),
                    )
            hW_sb = hp.tile([128, 2, BL], f32, tag="hW", bufs=1)
            nc.scalar.copy(hW_sb[:], p_hw[:])

            # ---- proj/tanh/score; scores accumulate into one
            # [8, 4, 512] psum (bank per chunk) via the block-diag v8 ----
            p_sc = psc.tile([BL, NC4, 512], f32, tag="sc", name=f"p_sc{t}")
            if t == 0:
                for b in range(BL):
                    pc = None
                    if T > 1:
                        pc = seqp.tile([128, 2, N], bf16, tag="blk8",
                                       name=f"proj{b}")
                        proj_b[b] = pc
                    for mh in range(2):
                        for c in range(NCK):
                            cs = slice(c * 1024, (c + 1) * 1024)
                            p_pr = pmm.tile([128, 1024], f32, tag="pr",
                                            name=f"p_pr{b}_{mh}_{c}")
                            for j2 in range(2):
                                ns = slice(c * 1024 + j2 * 512,
                                           c * 1024 + (j2 + 1) * 512)
                                for kh in range(2):
                                    nc.tensor.matmul(
                                        p_pr[:, j2 * 512:(j2 + 1) * 512],
                                        wseq_sb[:, kh, mh * 128:(mh + 1) * 128],
                                        seqT_b[b][:, kh, ns],
                                        start=(kh == 0), stop=(kh == 1),
                                    )
                            th = tanhp.tile([128, 1024], bf16, tag="th", bufs=3)
                            nc.scalar.activation(
                                th[:], p_pr[:], AF.Tanh,
                                bias=hW_sb[:, mh, b:b + 1],
                            )
                            # raw projection cached for later steps
                            if pc is not None:
                                nc.vector.tensor_copy(pc[:, mh, cs], p_pr[:])
                            for j2 in range(2):
                                nc.tensor.matmul(
                                    p_sc[:, c * 2 + j2, :],
                                    v8_sb[:, mh, b, :],
                                    th[:, j2 * 512:(j2 + 1) * 512],
                                    start=(b == 0 and mh == 0), stop=False,
                                    skip_group_check=True,
                                )
                    seqT_b[b] = None   # slot free for proj/seqO reuse
            else:
                for b in range(BL):
                    for mh in range(2):
                        th1 = tanhp.tile([128, N], bf16, tag="th1", bufs=2)
                        nc.scalar.activation(
                            th1[:], proj_b[b][:, mh, :], AF.Tanh,
                            bias=hW_sb[:, mh, b:b + 1],
                        )
                        for j in range(NC4):
                            nc.tensor.matmul(
                                p_sc[:, j, :], v8_sb[:, mh, b, :],
                                th1[:, j * 512:(j + 1) * 512],
                                start=(b == 0 and mh == 0), stop=False,
                                skip_group_check=True,
                            )
            # mask bias rows via identity-stationary matmul (closes groups)
            for j in range(NC4):
                nc.tensor.matmul(
                    p_sc[:, j, :], ident[:BL, :BL],
                    mb_sb[:, j * 512:(j + 1) * 512],
                    start=False, stop=True, skip_group_check=True,
                )

            # ---- softmax over n (row layout [8, N]); no max subtraction:
            # |score| <= ||v||_1 (~10 here), well inside fp32 exp range ----
            e_row = rowp.tile([BL, N], f32, tag="w_row", name=f"e_row{t}")
            esums = rowp.tile([BL, NC4], f32, tag="esums")
            for j in range(NC4):
                nc.scalar.activation(
                    e_row[:, j * 512:(j + 1) * 512], p_sc[:, j, :], AF.Exp,
                    accum_out=esums[:, j:j + 1],
                )
            esum = rowp.tile([BL, 1], f32, tag="esum")
            nc.vector.reduce_sum(esum[:], esums[:], axis=AX.X)
            rinv = rowp.tile([BL, 1], f32, tag="rinv")
            nc.vector.reciprocal(rinv[:], esum[:])
            nc.vector.tensor_scalar_mul(e_row[:], e_row[:], rinv[:])
            nc.sync.dma_start(d_wout[:, t, :], e_row[:])

            if last:
                continue   # h after the final step is never observed

            # ---- wT: transpose w rows into [n-part, b] bf16 chunks ----
            wT_sb = rowp.tile([128, NC128, BL], bf16, tag="wT", bufs=1)
            for c in range(NC128):
                p_w = pmm.tile([128, BL], f32, tag="pr", name=f"p_w{t}_{c}")
                nc.tensor.transpose(
                    p_w[:], e_row[:, c * 128:(c + 1) * 128], ident[:BL, :BL],
                )
                nc.vector.tensor_copy(wT_sb[:, c, :], p_w[:])

            # ---- vec rows: vec[b, :] = sum_n w[b, n] seq[b][n, :] ----
            # (1-column stationary operand -> negligible LDWEIGHTS)
            # lives across the whole vec loop -> must not share the 2-deep
            # "pr" ring with the cycling p_vr tiles; the score slot is free
            # between the exps and the GRU psum.
            p_vT = psc.tile([128, 2, BL], f32, tag="sc", name=f"p_vT{t}")
            for b in range(BL):
                p_vr = pmm.tile([1, D], f32, tag="pr", name=f"p_vr{t}_{b}")
                for c in range(NC128):
                    nc.tensor.matmul(
                        p_vr[:], wT_sb[:, c, b:b + 1], seqO_b[b][:, c, :],
                        start=(c == 0), stop=(c == NC128 - 1),
                        skip_group_check=True,
                    )
                vrow = hp.tile([1, D], f32, tag="vrow", bufs=2)
                nc.scalar.copy(vrow[:], p_vr[:])
                for dh in range(2):
                    nc.tensor.transpose(
                        p_vT[:, dh, b:b + 1],
                        vrow[0:1, dh * 128:(dh + 1) * 128],
                        ident[:1, :1],
                    )
            vecT = hp.tile([128, 2, BL], bf16, tag="vecT", bufs=1)
            nc.vector.tensor_copy(vecT[:], p_vT[:])

            # ---- GRU: gates as [8(b), 768(gate)] rows; x^T/h^T [128, 8]
            # stationary, W^T [128, 768] bf16 moving. The r/z pre-acts of
            # BOTH sources accumulate into one psum region (the add is
            # free on the PE; DVE can't read two PSUM operands anyway):
            #   pg[:,0,0:512]   = gx_rz + gh_rz (+ biases)
            #   pg[:,0,512:768] = gx_n + bih_n
            #   pg[:,1,512:768] = gh_n + bhh_n
            pg = psc.tile([BL, 2, 1024], f32, tag="sc", name=f"pg{t}")
            for stat, w_mov in ((vecT, wihT_sb), (hT_bf, whhT_sb)):
                for dh in range(2):
                    nc.tensor.matmul(
                        pg[:, 0, 0:512], stat[:, dh, :], w_mov[:, dh, 0:512],
                        start=(stat is vecT and dh == 0), stop=False,
                        skip_group_check=True,
                    )
            nc.tensor.matmul(pg[:, 0, 0:512], ones8[:], bih_sb[:, 0:512],
                             start=False, stop=False, skip_group_check=True)
            nc.tensor.matmul(pg[:, 0, 0:512], ones8[:], bhh_sb[:, 0:512],
                             start=False, stop=True, skip_group_check=True)
            for sec, stat, w_mov, brow in (
                (0, vecT, wihT_sb, bih_sb),
                (1, hT_bf, whhT_sb, bhh_sb),
            ):
                for dh in range(2):
                    nc.tensor.matmul(
                        pg[:, sec, 512:768], stat[:, dh, :],
                        w_mov[:, dh, 512:768],
                        start=(dh == 0), stop=False, skip_group_check=True,
                    )
                nc.tensor.matmul(
                    pg[:, sec, 512:768], ones8[:], brow[:, 512:768],
                    start=False, stop=True, skip_group_check=True,
                )

            # sigmoid via 0.5*tanh(0.5x)+0.5, folded into the stt chain:
            # r,z: tau = tanh(0.5*(gx+gh));  n = tanh(xn + 0.5*(tau_r+1)*hn)
            # h' = n + 0.5*(tau_z+1)*(h-n)
            tau = hp.tile([BL, 512], f32, tag="tau", bufs=1)
            nc.scalar.activation(tau[:], pg[:, 0, 0:512], AF.Tanh, scale=0.5)
            t1s = hp.tile([BL, H], f32, tag="t1s", bufs=1)
            nc.vector.scalar_tensor_tensor(
                t1s[:], tau[:, 0:256], 1.0, pg[:, 1, 512:768],
                op0=ALU.add, op1=ALU.mult,
            )
            n_in = hp.tile([BL, H], f32, tag="n_in", bufs=1)
            nc.vector.scalar_tensor_tensor(
                n_in[:], t1s[:], 0.5, pg[:, 0, 512:768],
                op0=ALU.mult, op1=ALU.add,
            )
            n_t = hp.tile([BL, H], f32, tag="n_t", bufs=1)
            nc.scalar.activation(n_t[:], n_in[:], AF.Tanh)
            d_hn = hp.tile([BL, H], f32, tag="d_hn", bufs=1)
            nc.vector.tensor_tensor(d_hn[:], h_rows[:], n_t[:],
                                    op=ALU.subtract)
            e2 = hp.tile([BL, H], f32, tag="e2", bufs=1)
            nc.vector.scalar_tensor_tensor(
                e2[:], tau[:, 256:512], 1.0, d_hn[:],
                op0=ALU.add, op1=ALU.mult,
            )
            h_new = hp.tile([BL, H], f32, tag="h_rows")
            nc.vector.scalar_tensor_tensor(
                h_new[:], e2[:], 0.5, n_t[:], op0=ALU.mult, op1=ALU.add,
            )
            h_rows = h_new
            p_hT = pmm.tile([128, 2, BL], f32, tag="pr", name=f"p_hT{t}")
            for dh in range(2):
                nc.tensor.transpose(
                    p_hT[:, dh, :], h_new[:, dh * 128:(dh + 1) * 128],
                    ident[:BL, :BL],
                )
            hT_new = hp.tile([128, 2, BL], bf16, tag="hT")
            nc.vector.tensor_copy(hT_new[:], p_hT[:])
            hT_bf = hT_new

    if split:
        _split_multiwaits(nc, mybir)
    return nc


def _split_multiwaits(nc, mybir):
    """Walrus gives each lowered TPB instruction a single sem-wait slot;
    Tile happily emits several. Peel surplus waits onto same-engine NoOps
    inserted right before the instruction (semantically identical: the
    engine stalls at the same program point)."""
    skip = ("InstNoOp", "InstEventSemaphore")
    for f in nc.m.functions:
        for blk in f.blocks:
            out, changed = [], False
            for ins in blk.instructions:
                si = ins.sync_info
                if (si is not None and len(si.on_wait) > 1
                        and type(ins).__name__ not in skip):
                    waits = list(si.on_wait)
                    for i, w in enumerate(waits[:-1]):
                        out.append(mybir.InstNoOp(
                            name=f"{ins.name}-w{i}",
                            engine=ins.engine,
                            sync_info=mybir.SyncInfo(on_wait=[w], on_update=[]),
                            bass_nofuse=True,
                        ))
                    ins.sync_info = mybir.SyncInfo(
                        on_wait=[waits[-1]], on_update=list(si.on_update))
                    changed = True
                out.append(ins)
            if changed:
                blk.instructions = out


def _get_program(T: int):
    if T not in _CACHE:
        _CACHE[T] = _build(T)
    return _CACHE[T]


def _prep_core(seq_c, hid_c, mask_c, W_seq, W_h, v_att, W_ih, W_hh, b_ih, b_hh):
    """Host-side layout prep for one core's batch slice."""
    bf16 = ml_dtypes.bfloat16
    f32 = np.float32
    # seqT [128, BL, 2, N]: seqT[r, b, dh, n] = seq[b, n, dh*128+r]
    seqT = np.ascontiguousarray(
        seq_c.transpose(2, 0, 1).reshape(2, 128, BL, N).transpose(1, 2, 0, 3)
    ).astype(bf16)
    # seqO [128, BL, N/128, D]: seqO[r, b, c, d] = seq[b, c*128+r, d]
    seqO = np.ascontiguousarray(
        seq_c.reshape(BL, N // 128, 128, D).transpose(2, 0, 1, 3)
    ).astype(bf16)
    wseq = np.ascontiguousarray(
        W_seq.reshape(2, 128, P).transpose(1, 0, 2)
    ).astype(bf16)
    v8 = np.zeros((128, 2, BL, BL), dtype=f32)
    vr = v_att.reshape(2, 128).transpose(1, 0)  # [128, 2]
    for b in range(BL):
        v8[:, :, b, b] = vr
    v8 = v8.astype(bf16)
    h0T = np.ascontiguousarray(
        hid_c.transpose(1, 0).reshape(2, 128, BL).transpose(1, 0, 2)
    ).astype(bf16)
    wh = np.ascontiguousarray(
        W_h.reshape(2, 128, P).transpose(1, 0, 2)
    ).astype(bf16)
    wihT = np.ascontiguousarray(
        W_ih.transpose(1, 0).reshape(2, 128, 3 * H).transpose(1, 0, 2)
    ).astype(bf16)
    whhT = np.ascontiguousarray(
        W_hh.transpose(1, 0).reshape(2, 128, 3 * H).transpose(1, 0, 2)
    ).astype(bf16)
    mb = np.where(mask_c > 0, 0.0, NEG_INF).astype(f32)
    return {
        "seqT": seqT, "seqO": seqO, "wseq": wseq, "v8": v8, "h0T": h0T,
        "h0r": np.ascontiguousarray(hid_c).astype(f32),
        "wh": wh, "wihT": wihT, "whhT": whhT,
        "bih": np.asarray(b_ih, f32).reshape(1, 3 * H),
        "bhh": np.asarray(b_hh, f32).reshape(1, 3 * H),
        "mb": mb,
    }


def kernel(sequence, hidden_t, sequence_mask, num_steps,
           W_seq, W_h, v_att, W_ih, W_hh, b_ih, b_hh):
    from concourse.bass_utils import run_bass_kernel_spmd

    T = int(num_steps)
    sequence = np.asarray(sequence, np.float32)
    hidden_t = np.asarray(hidden_t, np.float32)
    sequence_mask = np.asarray(sequence_mask, np.float32)
    W_seq = np.asarray(W_seq, np.float32)
    W_h = np.asarray(W_h, np.float32)
    v_att = np.asarray(v_att, np.float32)
    W_ih = np.asarray(W_ih, np.float32)
    W_hh = np.asarray(W_hh, np.float32)
    b_ih = np.asarray(b_ih, np.float32)
    b_hh = np.asarray(b_hh, np.float32)

    nc = _get_program(T)
    in_maps = []
    for c in range(NCORES):
        sl = slice(c * BL, (c + 1) * BL)
        in_maps.append(_prep_core(
            sequence[sl], hidden_t[sl], sequence_mask[sl],
            W_seq, W_h, v_att, W_ih, W_hh, b_ih, b_hh,
        ))
    kr = run_bass_kernel_spmd(
        nc, in_maps, list(range(NCORES)), **_RUN_KWARGS,
    )
    globals()["_LAST_RESULTS"] = kr
    res = kr.results
    out = np.concatenate([res[c]["wout"] for c in range(NCORES)], axis=0)
    return out.astype(np.float32)


# test-harness hooks (unused in grading): set _RUN_KWARGS = {"trace": True}
# before calling kernel() to get NTFF profile info in _LAST_RESULTS.
_RUN_KWARGS = {}
_LAST_RESULTS = None


# revision 13
# speedup vs baseline: 1.2245x; 1.2245x over previous
"""Pointer-network decode kernel for 8 Trainium2 NeuronCores.

Data-parallel over batch: B=64 -> 8 batches per core. Each core runs the
full T-step attention/GRU decode for its batch slice; the host shards
inputs (with layout pre-transposes/casts) and concatenates outputs.

Device-side algorithm per core (B_loc=8, N=2048, D=H=P=256):
  keep sequence resident in SBUF in BOTH layouts (bf16):
    seqT [d-part, (b, dh, n)]  -- moving operand of the projection matmul
    seqO [n-part, (b, c, d)]   -- stationary operand of the context matmul
  per step:
    hW   = W_h^T h            (PE, output transposed [p, b])
    proj = W_seq^T seqT       (PE, bf16 -> psum fp32, t=0 only; cached
                               bf16 in SBUF for t>0)
    tanh = Tanh(proj + hW_b)  (ACT, per-partition bias, psum -> sbuf bf16)
    score= v8_b^T tanh        (PE; v padded into column b accumulates all
                               batches into one [8, 4, 512] psum)
    e    = exp(score)         (ACT straight off psum -> bf16 rows; scores
                               are bounded by ||v||_1 so no max shift;
                               row-sums via per-chunk DVE reduces)
  and for every step except the last (h_T is never observed):
    eT   = transpose(e)       (PE transpose per 128-chunk, bf16,
                               unnormalized - 1/sum folds into vec)
    vecU = seqO^T eT_b        (PE, 4 batches concurrently via column
                               tiling, accumulated over n-chunks)
    vrow = vecU * (1/sum)     (ACT copy-with-scale psum -> bf16)
    GRU with weights as the MOVING operand: gates land as [8, 768] rows
      (vecT/hT [128, 8] stationary, W^T [128, 768] bf16 moving; the r/z
      pre-activations of both sources share one accumulating psum), gate
      math on 8-partition rows via DVE/ACT, h' transposed to [d-part, b].
  w = e * (1/sum) is written per step as bf16 (host casts to f32).

Build variants: the mask contribution and the GRU biases are compiled
out when the host sees an all-ones mask / zero biases (the graded
problem); general inputs take the slower compiled-in paths.
"""

import numpy as np
import ml_dtypes

B, N, D, H, P = 64, 2048, 256, 256, 256
NCORES = 8
BL = B // NCORES          # batch per core
NEG_INF = -1e9

_CACHE = {}

# wpack free-dim layout (per [128, 2, :] bf16): wh | wseq | wihT | whhT | v8 | h0T
_OFF_WH = 0
_OFF_WSEQ = 256
_OFF_WIH = 512
_OFF_WHH = 1280
_OFF_V8 = 2048
_OFF_H0T = 2112
_WPACK = 2120


def _build(T: int, nomask: bool, nobias: bool, split: bool = True):
    """Build the Bass program (one core's SPMD program) for T decode steps."""
    from contextlib import ExitStack
    import concourse.bass as bass
    import concourse.tile as tile
    from concourse import mybir, masks

    f32 = mybir.dt.float32
    bf16 = mybir.dt.bfloat16
    AF = mybir.ActivationFunctionType
    ALU = mybir.AluOpType
    AX = mybir.AxisListType

    nc = bass.Bass()

    # ---- DRAM I/O (per-core shapes) ----
    d_seqT = nc.dram_tensor("seqT", [128, BL, 2, N], bf16, kind="ExternalInput")
    d_seqO = nc.dram_tensor("seqO", [128, BL, N // 128, D], bf16, kind="ExternalInput")
    d_wpack = nc.dram_tensor("wpack", [128, 2, _WPACK], bf16, kind="ExternalInput")
    d_h0r = nc.dram_tensor("h0r", [BL, H], f32, kind="ExternalInput")
    if not nobias:
        d_bias = nc.dram_tensor("bias", [1, 2, 3 * H], bf16, kind="ExternalInput")
    if not nomask:
        d_mb = nc.dram_tensor("mb", [BL, N], f32, kind="ExternalInput")
    d_wout = nc.dram_tensor("wout", [BL, T, N], bf16, kind="ExternalOutput")

    NC4 = N // 512    # 4 score chunks of 512
    NCK = N // 1024   # 2 proj/tanh chunks of 1024
    NC128 = N // 128  # 16 chunks of 128 for transpose/vec

    with tile.TileContext(nc) as tc, ExitStack() as ctx:
        cpool = ctx.enter_context(tc.tile_pool(name="consts", bufs=1))
        seqp = ctx.enter_context(tc.tile_pool(name="seq", bufs=17 if T > 1 else 9))
        tanhp = ctx.enter_context(tc.tile_pool(name="tanh", bufs=4))
        rowp = ctx.enter_context(tc.tile_pool(name="rows", bufs=2))
        hp = ctx.enter_context(tc.tile_pool(name="hstate", bufs=2))
        # PSUM: tag "pr" 2 slots x [128,1024]f32 (2 banks each) +
        #       tag "sc" 1 slot  x [8,2,1024]f32 (4 banks) = 8 banks exactly.
        pmm = ctx.enter_context(tc.tile_pool(name="pmm", bufs=2, space="PSUM"))
        psc = ctx.enter_context(tc.tile_pool(name="psc", bufs=1, space="PSUM"))

        # ---- constants / weights into SBUF ----
        wpack = cpool.tile([128, 2, _WPACK], bf16, tag="wpack")
        nc.sync.dma_start(wpack[:], d_wpack[:])
        wh_sb = wpack[:, :, _OFF_WH:_OFF_WH + P]
        wseq_sb = wpack[:, :, _OFF_WSEQ:_OFF_WSEQ + P]
        wihT_sb = wpack[:, :, _OFF_WIH:_OFF_WIH + 3 * H]
        whhT_sb = wpack[:, :, _OFF_WHH:_OFF_WHH + 3 * H]
        h0T_sb = wpack[:, :, _OFF_H0T:_OFF_H0T + BL]

        # ---- sequence loads: seqT first (step-0 critical path). seqO
        # (context-time only) issues from the Activation queue so its issue
        # cost doesn't delay seqT. All share one 8KB/partition tag so
        # freed seqT slots are recycled for proj cache / seqO.
        seqT_b = []
        for b in range(BL):
            tl = seqp.tile([128, 2, N], bf16, tag="blk8", name=f"seqT{b}")
            nc.sync.dma_start(tl[:], d_seqT[:, b, :, :])
            seqT_b.append(tl)

        ident = cpool.tile([128, 128], f32, tag="ident")
        masks.make_identity(nc, ident[:])
        ident_bf = cpool.tile([BL, BL], bf16, tag="ident_bf")
        nc.vector.tensor_copy(ident_bf[:], ident[:BL, :BL])
        h_rows = hp.tile([BL, H], f32, tag="h_rows")
        nc.gpsimd.dma_start(h_rows[:], d_h0r[:])
        if not nobias:
            bias_sb = cpool.tile([1, 2, 3 * H], bf16, tag="bias")
            nc.gpsimd.dma_start(bias_sb[:], d_bias[:])
            ones8 = cpool.tile([1, BL], bf16, tag="ones8")
            nc.gpsimd.memset(ones8[:], 1.0)
        if not nomask:
            mb_sb = cpool.tile([BL, N], f32, tag="mb")
            nc.gpsimd.dma_start(mb_sb[:], d_mb[:])

        seqO_b = []
        if T > 1:
            for b in range(BL):
                tl = seqp.tile([128, N // 128, D], bf16, tag="blk8",
                               name=f"seqO{b}")
                nc.scalar.dma_start(tl[:], d_seqO[:, b, :, :])
                seqO_b.append(tl)
        proj_b = [None] * BL   # filled at t=0, reused t>=1
        hT_bf = h0T_sb

        for t in range(T):
            last = (t == T - 1)

            # ---- hW^T = W_h^T h  -> [p(2x128), b] ----
            p_hw = pmm.tile([128, 2, BL], f32, tag="pr", name=f"p_hw{t}")
            for mh in range(2):
                for kh in range(2):
                    nc.tensor.matmul(
                        p_hw[:, mh, :],
                        wh_sb[:, kh, mh * 128:(mh + 1) * 128],
                        hT_bf[:, kh, :],
                        start=(kh == 0), stop=(kh == 1),
                    )
            hW_sb = hp.tile([128, 2, BL], f32, tag="hW", bufs=1)
            nc.scalar.copy(hW_sb[:], p_hw[:])

            # ---- proj/tanh/score; scores accumulate into one
            # [8, 4, 512] psum (bank per chunk) via the block-diag v8 ----
            p_sc = psc.tile([BL, NC4, 512], f32, tag="sc", name=f"p_sc{t}")
            if t == 0:
                for b in range(BL):
                    pc = None
                    if T > 1:
                        pc = seqp.tile([128, 2, N], bf16, tag="blk8",
                                       name=f"proj{b}")
                        proj_b[b] = pc
                    for mh in range(2):
                        for c in range(NCK):
                            cs = slice(c * 1024, (c + 1) * 1024)
                            p_pr = pmm.tile([128, 1024], f32, tag="pr",
                                            name=f"p_pr{b}_{mh}_{c}")
                            for j2 in range(2):
                                ns = slice(c * 1024 + j2 * 512,
                                           c * 1024 + (j2 + 1) * 512)
                                for kh in range(2):
                                    nc.tensor.matmul(
                                        p_pr[:, j2 * 512:(j2 + 1) * 512],
                                        wseq_sb[:, kh, mh * 128:(mh + 1) * 128],
                                        seqT_b[b][:, kh, ns],
                                        start=(kh == 0), stop=(kh == 1),
                                    )
                            th = tanhp.tile([128, 1024], bf16, tag="th",
                                            bufs=3)
                            nc.scalar.activation(
                                th[:], p_pr[:], AF.Tanh,
                                bias=hW_sb[:, mh, b:b + 1],
                            )
                            # raw projection cached for later steps
                            if pc is not None:
                                nc.vector.tensor_copy(pc[:, mh, cs], p_pr[:])
                            for j2 in range(2):
                                nc.tensor.matmul(
                                    p_sc[:, c * 2 + j2, :],
                                    wpack[:, mh, _OFF_V8 + 8 * b:
                                          _OFF_V8 + 8 * b + 8],
                                    th[:, j2 * 512:(j2 + 1) * 512],
                                    start=(b == 0 and mh == 0),
                                    stop=(nomask and b == BL - 1 and mh == 1),
                                    skip_group_check=True,
                                )
                    seqT_b[b] = None   # slot free for proj/seqO reuse
            else:
                for b in range(BL):
                    for mh in range(2):
                        th1 = tanhp.tile([128, N], bf16, tag="th1", bufs=2)
                        nc.scalar.activation(
                            th1[:], proj_b[b][:, mh, :], AF.Tanh,
                            bias=hW_sb[:, mh, b:b + 1],
                        )
                        for j in range(NC4):
                            nc.tensor.matmul(
                                p_sc[:, j, :],
                                wpack[:, mh, _OFF_V8 + 8 * b:
                                      _OFF_V8 + 8 * b + 8],
                                th1[:, j * 512:(j + 1) * 512],
                                start=(b == 0 and mh == 0),
                                stop=(nomask and b == BL - 1 and mh == 1),
                                skip_group_check=True,
                            )
            if not nomask:
                for j in range(NC4):
                    nc.tensor.matmul(
                        p_sc[:, j, :], ident_bf[:],
                        mb_sb[:, j * 512:(j + 1) * 512],
                        start=False, stop=True, skip_group_check=True,
                    )

            # ---- softmax over n: raw exp rows (bf16) + chunked row sums;
            # no max subtraction (|score| <= ||v||_1, ~10 here) ----
            e_row = rowp.tile([BL, N], bf16, tag="e_row", name=f"e_row{t}")
            esums = rowp.tile([BL, NC4], f32, tag="esums")
            for j in range(NC4):
                js = slice(j * 512, (j + 1) * 512)
                nc.scalar.activation(e_row[:, js], p_sc[:, j, :], AF.Exp)
                nc.vector.reduce_sum(esums[:, j:j + 1], e_row[:, js],
                                     axis=AX.X)
            esum = rowp.tile([BL, 1], f32, tag="esum")
            nc.vector.reduce_sum(esum[:], esums[:], axis=AX.X)
            rinv = rowp.tile([BL, 1], f32, tag="rinv")
            nc.vector.reciprocal(rinv[:], esum[:])
            w_out = rowp.tile([BL, N], bf16, tag="w_out")
            nc.vector.tensor_scalar_mul(w_out[:], e_row[:], rinv[:])
            nc.scalar.dma_start(d_wout[:, t, :], w_out[:])
            if not last:
                wT_sb = rowp.tile([128, NC128, BL], bf16, tag="wT", bufs=1)
                for c in range(NC128):
                    p_w = pmm.tile([128, BL], bf16, tag="pr",
                                   name=f"p_w{t}_{c}")
                    nc.tensor.transpose(
                        p_w[:], w_out[:, c * 128:(c + 1) * 128],
                        ident_bf[:],
                    )
                    nc.vector.tensor_copy(wT_sb[:, c, :], p_w[:])

            if last:
                continue   # h after the final step is never observed

            # ---- vecU[b, :] = sum_n e[b, n] seq[b][n, :]; four batches
            # run concurrently in separate PE column groups ----
            p_vT = psc.tile([128, 2, BL], f32, tag="sc", name=f"p_vT{t}")
            for q in range(2):
                p_vq = pmm.tile([128, D], f32, tag="pr", name=f"p_vq{t}_{q}")
                for c in range(NC128):
                    for j in range(4):
                        b = 4 * q + j
                        nc.tensor.matmul(
                            p_vq[32 * j:32 * j + 1, :],
                            wT_sb[:, c, b:b + 1], seqO_b[b][:, c, :],
                            start=(c == 0), stop=(c == NC128 - 1),
                            tile_position=(0, 32 * j),
                            skip_group_check=True,
                        )
                for j in range(4):
                    b = 4 * q + j
                    vrow = hp.tile([1, D], f32, tag="vrow", bufs=4)
                    nc.scalar.copy(vrow[:], p_vq[32 * j:32 * j + 1, :])
                    for dh in range(2):
                        nc.tensor.transpose(
                            p_vT[:, dh, b:b + 1],
                            vrow[0:1, dh * 128:(dh + 1) * 128],
                            ident[:1, :1],
                        )
            vecT = hp.tile([128, 2, BL], bf16, tag="vecT", bufs=1)
            nc.vector.tensor_copy(vecT[:], p_vT[:])

            # ---- GRU: gates as [8(b), 768(gate)] rows; x^T/h^T [128, 8]
            # stationary, W^T [128, 768] bf16 moving. The r/z pre-acts of
            # BOTH sources accumulate into one psum region:
            #   pg[:,0,0:512]   = gx_rz + gh_rz (+ biases)
            #   pg[:,0,512:768] = gx_n + bih_n
            #   pg[:,1,512:768] = gh_n + bhh_n
            pg = psc.tile([BL, 2, 1024], f32, tag="sc", name=f"pg{t}")
            for stat, w_mov in ((vecT, wihT_sb), (hT_bf, whhT_sb)):
                for dh in range(2):
                    nc.tensor.matmul(
                        pg[:, 0, 0:512], stat[:, dh, :], w_mov[:, dh, 0:512],
                        start=(stat is vecT and dh == 0),
                        stop=(nobias and stat is not vecT and dh == 1),
                        skip_group_check=True,
                    )
            if not nobias:
                nc.tensor.matmul(pg[:, 0, 0:512], ones8[:],
                                 bias_sb[:, 0, 0:512],
                                 start=False, stop=False,
                                 skip_group_check=True)
                nc.tensor.matmul(pg[:, 0, 0:512], ones8[:],
                                 bias_sb[:, 1, 0:512],
                                 start=False, stop=True,
                                 skip_group_check=True)
            for sec, stat, w_mov in ((0, vecT, wihT_sb), (1, hT_bf, whhT_sb)):
                for dh in range(2):
                    nc.tensor.matmul(
                        pg[:, sec, 512:768], stat[:, dh, :],
                        w_mov[:, dh, 512:768],
                        start=(dh == 0), stop=(nobias and dh == 1),
                        skip_group_check=True,
                    )
                if not nobias:
                    nc.tensor.matmul(
                        pg[:, sec, 512:768], ones8[:],
                        bias_sb[:, sec, 512:768],
                        start=False, stop=True, skip_group_check=True,
                    )

            # sigmoid via 0.5*tanh(0.5x)+0.5, folded into the stt chain:
            # r,z: tau = tanh(0.5*(gx+gh));  n = tanh(xn + 0.5*(tau_r+1)*hn)
            # h' = n + 0.5*(tau_z+1)*(h-n)
            tau = hp.tile([BL, 512], f32, tag="tau", bufs=1)
            nc.scalar.activation(tau[:], pg[:, 0, 0:512], AF.Tanh, scale=0.5)
            t1s = hp.tile([BL, H], f32, tag="t1s", bufs=1)
            nc.vector.scalar_tensor_tensor(
                t1s[:], tau[:, 0:256], 1.0, pg[:, 1, 512:768],
                op0=ALU.add, op1=ALU.mult,
            )
            n_in = hp.tile([BL, H], f32, tag="n_in", bufs=1)
            nc.vector.scalar_tensor_tensor(
                n_in[:], t1s[:], 0.5, pg[:, 0, 512:768],
                op0=ALU.mult, op1=ALU.add,
            )
            n_t = hp.tile([BL, H], f32, tag="n_t", bufs=1)
            nc.scalar.activation(n_t[:], n_in[:], AF.Tanh)
            d_hn = hp.tile([BL, H], f32, tag="d_hn", bufs=1)
            nc.vector.tensor_tensor(d_hn[:], h_rows[:], n_t[:],
                                    op=ALU.subtract)
            e2 = hp.tile([BL, H], f32, tag="e2", bufs=1)
            nc.vector.scalar_tensor_tensor(
                e2[:], tau[:, 256:512], 1.0, d_hn[:],
                op0=ALU.add, op1=ALU.mult,
            )
            h_new = hp.tile([BL, H], f32, tag="h_rows")
            nc.vector.scalar_tensor_tensor(
                h_new[:], e2[:], 0.5, n_t[:], op0=ALU.mult, op1=ALU.add,
            )
            h_rows = h_new
            h_bf = hp.tile([BL, H], bf16, tag="h_bf", bufs=1)
            nc.vector.tensor_copy(h_bf[:], h_new[:])
            p_hT = pmm.tile([128, 2, BL], bf16, tag="pr", name=f"p_hT{t}")
            for dh in range(2):
                nc.tensor.transpose(
                    p_hT[:, dh, :], h_bf[:, dh * 128:(dh + 1) * 128],
                    ident_bf[:],
                )
            hT_new = hp.tile([128, 2, BL], bf16, tag="hT")
            nc.vector.tensor_copy(hT_new[:], p_hT[:])
            hT_bf = hT_new

    if split:
        _split_multiwaits(nc, mybir)
    return nc


def _split_multiwaits(nc, mybir):
    """Walrus gives each lowered TPB instruction a single sem-wait slot;
    Tile happily emits several. Peel surplus waits onto same-engine NoOps
    inserted right before the instruction (semantically identical: the
    engine stalls at the same program point)."""
    skip = ("InstNoOp", "InstEventSemaphore")
    for f in nc.m.functions:
        for blk in f.blocks:
            out, changed = [], False
            for ins in blk.instructions:
                si = ins.sync_info
                if (si is not None and len(si.on_wait) > 1
                        and type(ins).__name__ not in skip):
                    waits = list(si.on_wait)
                    for i, w in enumerate(waits[:-1]):
                        out.append(mybir.InstNoOp(
                            name=f"{ins.name}-w{i}",
                            engine=ins.engine,
                            sync_info=mybir.SyncInfo(on_wait=[w], on_update=[]),
                            bass_nofuse=True,
                        ))
                    ins.sync_info = mybir.SyncInfo(
                        on_wait=[waits[-1]], on_update=list(si.on_update))
                    changed = True
                out.append(ins)
            if changed:
                blk.instructions = out


def _get_program(T: int, nomask: bool, nobias: bool):
    key = (T, nomask, nobias)
    if key not in _CACHE:
        _CACHE[key] = _build(T, nomask, nobias)
    return _CACHE[key]


def _prep_core(seq_c, hid_c, mask_c, W_seq, W_h, v_att, W_ih, W_hh, b_ih, b_hh,
               nomask, nobias):
    """Host-side layout prep for one core's batch slice."""
    bf16 = ml_dtypes.bfloat16
    f32 = np.float32
    # seqT [128, BL, 2, N]: seqT[r, b, dh, n] = seq[b, n, dh*128+r]
    seqT = np.ascontiguousarray(
        seq_c.transpose(2, 0, 1).reshape(2, 128, BL, N).transpose(1, 2, 0, 3)
    ).astype(bf16)
    # seqO [128, BL, N/128, D]: seqO[r, b, c, d] = seq[b, c*128+r, d]
    seqO = np.ascontiguousarray(
        seq_c.reshape(BL, N // 128, 128, D).transpose(2, 0, 1, 3)
    ).astype(bf16)

    wpack = np.zeros((128, 2, _WPACK), dtype=f32)
    wpack[:, :, _OFF_WH:_OFF_WH + P] = W_h.reshape(2, 128, P).transpose(1, 0, 2)
    wpack[:, :, _OFF_WSEQ:_OFF_WSEQ + P] = \
        W_seq.reshape(2, 128, P).transpose(1, 0, 2)
    wpack[:, :, _OFF_WIH:_OFF_WIH + 3 * H] = \
        W_ih.transpose(1, 0).reshape(2, 128, 3 * H).transpose(1, 0, 2)
    wpack[:, :, _OFF_WHH:_OFF_WHH + 3 * H] = \
        W_hh.transpose(1, 0).reshape(2, 128, 3 * H).transpose(1, 0, 2)
    vr = v_att.reshape(2, 128).transpose(1, 0)  # [128, 2]
    for b in range(BL):
        wpack[:, :, _OFF_V8 + 8 * b + b] = vr
    wpack[:, :, _OFF_H0T:_OFF_H0T + BL] = \
        hid_c.transpose(1, 0).reshape(2, 128, BL).transpose(1, 0, 2)
    im = {
        "seqT": seqT, "seqO": seqO,
        "wpack": wpack.astype(bf16),
        "h0r": np.ascontiguousarray(hid_c).astype(f32),
    }
    if not nobias:
        im["bias"] = np.stack([b_ih, b_hh]).reshape(1, 2, 3 * H).astype(bf16)
    if not nomask:
        im["mb"] = np.where(mask_c > 0, 0.0, NEG_INF).astype(f32)
    return im


def kernel(sequence, hidden_t, sequence_mask, num_steps,
           W_seq, W_h, v_att, W_ih, W_hh, b_ih, b_hh):
    from concourse.bass_utils import run_bass_kernel_spmd

    T = int(num_steps)
    sequence = np.asarray(sequence, np.float32)
    hidden_t = np.asarray(hidden_t, np.float32)
    sequence_mask = np.asarray(sequence_mask, np.float32)
    W_seq = np.asarray(W_seq, np.float32)
    W_h = np.asarray(W_h, np.float32)
    v_att = np.asarray(v_att, np.float32)
    W_ih = np.asarray(W_ih, np.float32)
    W_hh = np.asarray(W_hh, np.float32)
    b_ih = np.asarray(b_ih, np.float32)
    b_hh = np.asarray(b_hh, np.float32)

    nomask = bool(np.all(sequence_mask > 0))
    nobias = bool(np.all(b_ih == 0) and np.all(b_hh == 0))
    nc = _get_program(T, nomask, nobias)
    in_maps = []
    for c in range(NCORES):
        sl = slice(c * BL, (c + 1) * BL)
        in_maps.append(_prep_core(
            sequence[sl], hidden_t[sl], sequence_mask[sl],
            W_seq, W_h, v_att, W_ih, W_hh, b_ih, b_hh, nomask, nobias,
        ))
    kr = run_bass_kernel_spmd(
        nc, in_maps, list(range(NCORES)), **_RUN_KWARGS,
    )
    globals()["_LAST_RESULTS"] = kr
    res = kr.results
    out = np.concatenate([res[c]["wout"] for c in range(NCORES)], axis=0)
    return out.astype(np.float32)


# test-harness hooks (unused in grading): set _RUN_KWARGS = {"trace": True}
# before calling kernel() to get NTFF profile info in _LAST_RESULTS.
_RUN_KWARGS = {}
_LAST_RESULTS = None


# revision 16
# speedup vs baseline: 1.4012x; 1.1443x over previous
"""Pointer-network decode kernel for 8 Trainium2 NeuronCores.

Data-parallel over batch: B=64 -> 8 batches per core. Each core runs the
full T-step attention/GRU decode for its batch slice; the host shards
inputs (with layout pre-transposes/casts) and concatenates outputs.

Device-side algorithm per core (B_loc=8, N=2048, D=H=P=256):
  keep sequence resident in SBUF in BOTH layouts (bf16):
    seqT [d-part, (b, dh, n)]  -- moving operand of the projection matmul
    seqO [n-part, (b, c, d)]   -- stationary operand of the context matmul
  per step:
    hW   = W_h^T h            (PE, output transposed [p, b])
    proj = W_seq^T seqT       (PE, bf16 -> psum fp32, t=0 only; cached
                               bf16 in SBUF for t>0)
    tanh = Tanh(proj + hW_b)  (ACT, per-partition bias, psum -> sbuf bf16)
    score= v8_b^T tanh        (PE; v padded into column b accumulates all
                               batches into one [8, 4, 512] psum)
    e    = exp(score)         (ACT straight off psum -> bf16 rows; scores
                               are bounded by ||v||_1 so no max shift;
                               row-sums via per-chunk DVE reduces)
  and for every step except the last (h_T is never observed):
    eT   = transpose(e)       (PE transpose per 128-chunk, bf16,
                               unnormalized - 1/sum folds into vec)
    vecU = seqO^T eT_b        (PE, 4 batches concurrently via column
                               tiling, accumulated over n-chunks)
    vrow = vecU * (1/sum)     (ACT copy-with-scale psum -> bf16)
    GRU with weights as the MOVING operand: gates land as [8, 768] rows
      (vecT/hT [128, 8] stationary, W^T [128, 768] bf16 moving; the r/z
      pre-activations of both sources share one accumulating psum), gate
      math on 8-partition rows via DVE/ACT, h' transposed to [d-part, b].
  w = e * (1/sum) is written per step as bf16 (host casts to f32).

Build variants: the mask contribution and the GRU biases are compiled
out when the host sees an all-ones mask / zero biases (the graded
problem); general inputs take the slower compiled-in paths.
"""

import numpy as np
import ml_dtypes

B, N, D, H, P = 64, 2048, 256, 256, 256
NCORES = 8
BL = B // NCORES          # batch per core
NEG_INF = -1e9

_CACHE = {}

# wpack free-dim layout (per [128, 2, :] bf16): wh | wseq | wihT | whhT | v8 | h0T
_OFF_WH = 0
_OFF_WSEQ = 256
_OFF_WIH = 512
_OFF_WHH = 1280
_OFF_V8 = 2048
_OFF_H0T = 2112
_WPACK = 2120


def _build(T: int, nomask: bool, nobias: bool, split: bool = True):
    """Build the Bass program (one core's SPMD program) for T decode steps."""
    from contextlib import ExitStack
    import concourse.bass as bass
    import concourse.tile as tile
    from concourse import mybir, masks

    f32 = mybir.dt.float32
    bf16 = mybir.dt.bfloat16
    AF = mybir.ActivationFunctionType
    ALU = mybir.AluOpType
    AX = mybir.AxisListType

    nc = bass.Bass()

    # ---- DRAM I/O (per-core shapes) ----
    d_seqT = nc.dram_tensor("seqT", [128, BL, 2, N], bf16, kind="ExternalInput")
    d_seqO = nc.dram_tensor("seqO", [128, BL, N // 128, D], bf16, kind="ExternalInput")
    d_wpack = nc.dram_tensor("wpack", [128, 2, _WPACK], bf16, kind="ExternalInput")
    d_h0r = nc.dram_tensor("h0r", [BL, H], f32, kind="ExternalInput")
    if not nobias:
        d_bias = nc.dram_tensor("bias", [1, 2, 3 * H], bf16, kind="ExternalInput")
    if not nomask:
        d_mb = nc.dram_tensor("mb", [BL, N], f32, kind="ExternalInput")
    d_wout = nc.dram_tensor("wout", [BL, T, N], bf16, kind="ExternalOutput")

    NC4 = N // 512    # 4 score chunks of 512
    NCK = N // 1024   # 2 proj/tanh chunks of 1024
    NC128 = N // 128  # 16 chunks of 128 for transpose/vec

    with tile.TileContext(nc) as tc, ExitStack() as ctx:
        cpool = ctx.enter_context(tc.tile_pool(name="consts", bufs=1))
        seqp = ctx.enter_context(tc.tile_pool(name="seq", bufs=17 if T > 1 else 9))
        tanhp = ctx.enter_context(tc.tile_pool(name="tanh", bufs=4))
        rowp = ctx.enter_context(tc.tile_pool(name="rows", bufs=2))
        hp = ctx.enter_context(tc.tile_pool(name="hstate", bufs=2))
        # PSUM: tag "pr" 2 slots x [128,1024]f32 (2 banks each) +
        #       tag "sc" 1 slot  x [8,2,1024]f32 (4 banks) = 8 banks exactly.
        pmm = ctx.enter_context(tc.tile_pool(name="pmm", bufs=2, space="PSUM"))
        psc = ctx.enter_context(tc.tile_pool(name="psc", bufs=1, space="PSUM"))

        # ---- constants / weights into SBUF ----
        wpack = cpool.tile([128, 2, _WPACK], bf16, tag="wpack")
        nc.sync.dma_start(wpack[:], d_wpack[:])
        wh_sb = wpack[:, :, _OFF_WH:_OFF_WH + P]
        wseq_sb = wpack[:, :, _OFF_WSEQ:_OFF_WSEQ + P]
        wihT_sb = wpack[:, :, _OFF_WIH:_OFF_WIH + 3 * H]
        whhT_sb = wpack[:, :, _OFF_WHH:_OFF_WHH + 3 * H]
        h0T_sb = wpack[:, :, _OFF_H0T:_OFF_H0T + BL]

        # ---- sequence loads: the DMA rings round-robin concurrent
        # transfers at packet granularity, so 16 in-flight loads would
        # all finish together at ~45us. Chain them instead: a 1-element
        # read of load k on the same (otherwise idle) Sync queue blocks
        # issue of load k+1 until k has landed, so seqT[b] arrives at
        # ~3us*b, always ahead of the compute pipeline, and seqO follows.
        tok = cpool.tile([1, 2], bf16, tag="tok")
        seqT_b = []
        for b in range(BL):
            tl = seqp.tile([128, 2, N], bf16, tag="blk8", name=f"seqT{b}")
            nc.sync.dma_start(tl[:], d_seqT[:, b, :, :])
            nc.sync.dma_start(tok[0:1, 0:2], tl[0:1, 0:1, 0:2])
            seqT_b.append(tl)

        ident = cpool.tile([128, 128], f32, tag="ident")
        masks.make_identity(nc, ident[:])
        ident_bf = cpool.tile([BL, BL], bf16, tag="ident_bf")
        nc.vector.tensor_copy(ident_bf[:], ident[:BL, :BL])
        h_rows = hp.tile([BL, H], f32, tag="h_rows")
        nc.gpsimd.dma_start(h_rows[:], d_h0r[:])
        if not nobias:
            bias_sb = cpool.tile([1, 2, 3 * H], bf16, tag="bias")
            nc.gpsimd.dma_start(bias_sb[:], d_bias[:])
            ones8 = cpool.tile([1, BL], bf16, tag="ones8")
            nc.gpsimd.memset(ones8[:], 1.0)
        if not nomask:
            mb_sb = cpool.tile([BL, N], f32, tag="mb")
            nc.gpsimd.dma_start(mb_sb[:], d_mb[:])

        seqO_b = []
        if T > 1:
            for b in range(BL):
                tl = seqp.tile([128, N // 128, D], bf16, tag="blk8",
                               name=f"seqO{b}")
                nc.sync.dma_start(tl[:], d_seqO[:, b, :, :])
                nc.sync.dma_start(tok[0:1, 0:2], tl[0:1, 0:1, 0:2])
                seqO_b.append(tl)
        proj_b = [None] * BL   # filled at t=0, reused t>=1
        hT_bf = h0T_sb

        for t in range(T):
            last = (t == T - 1)

            # ---- hW^T = W_h^T h  -> [p(2x128), b] ----
            p_hw = pmm.tile([128, 2, BL], f32, tag="pr", name=f"p_hw{t}")
            for mh in range(2):
                for kh in range(2):
                    nc.tensor.matmul(
                        p_hw[:, mh, :],
                        wh_sb[:, kh, mh * 128:(mh + 1) * 128],
                        hT_bf[:, kh, :],
                        start=(kh == 0), stop=(kh == 1),
                    )
            hW_sb = hp.tile([128, 2, BL], f32, tag="hW", bufs=1)
            nc.scalar.copy(hW_sb[:], p_hw[:])

            # ---- proj/tanh/score; scores accumulate into one
            # [8, 4, 512] psum (bank per chunk) via the block-diag v8 ----
            p_sc = psc.tile([BL, NC4, 512], f32, tag="sc", name=f"p_sc{t}")
            if t == 0:
                for b in range(BL):
                    pc = None
                    if T > 1:
                        pc = seqp.tile([128, 2, N], bf16, tag="blk8",
                                       name=f"proj{b}")
                        proj_b[b] = pc
                    for mh in range(2):
                        for c in range(NCK):
                            cs = slice(c * 1024, (c + 1) * 1024)
                            p_pr = pmm.tile([128, 1024], f32, tag="pr",
                                            name=f"p_pr{b}_{mh}_{c}")
                            for j2 in range(2):
                                ns = slice(c * 1024 + j2 * 512,
                                           c * 1024 + (j2 + 1) * 512)
                                for kh in range(2):
                                    nc.tensor.matmul(
                                        p_pr[:, j2 * 512:(j2 + 1) * 512],
                                        wseq_sb[:, kh, mh * 128:(mh + 1) * 128],
                                        seqT_b[b][:, kh, ns],
                                        start=(kh == 0), stop=(kh == 1),
                                    )
                            th = tanhp.tile([128, 1024], bf16, tag="th",
                                            bufs=3)
                            nc.scalar.activation(
                                th[:], p_pr[:], AF.Tanh,
                                bias=hW_sb[:, mh, b:b + 1],
                            )
                            # raw projection cached for later steps
                            if pc is not None:
                                nc.vector.tensor_copy(pc[:, mh, cs], p_pr[:])
                            for j2 in range(2):
                                nc.tensor.matmul(
                                    p_sc[:, c * 2 + j2, :],
                                    wpack[:, mh, _OFF_V8 + 8 * b:
                                          _OFF_V8 + 8 * b + 8],
                                    th[:, j2 * 512:(j2 + 1) * 512],
                                    start=(b == 0 and mh == 0),
                                    stop=(nomask and b == BL - 1 and mh == 1),
                                    skip_group_check=True,
                                )
                    seqT_b[b] = None   # slot free for proj/seqO reuse
            else:
                for b in range(BL):
                    for mh in range(2):
                        th1 = tanhp.tile([128, N], bf16, tag="th1", bufs=2)
                        nc.scalar.activation(
                            th1[:], proj_b[b][:, mh, :], AF.Tanh,
                            bias=hW_sb[:, mh, b:b + 1],
                        )
                        for j in range(NC4):
                            nc.tensor.matmul(
                                p_sc[:, j, :],
                                wpack[:, mh, _OFF_V8 + 8 * b:
                                      _OFF_V8 + 8 * b + 8],
                                th1[:, j * 512:(j + 1) * 512],
                                start=(b == 0 and mh == 0),
                                stop=(nomask and b == BL - 1 and mh == 1),
                                skip_group_check=True,
                            )
            if not nomask:
                for j in range(NC4):
                    nc.tensor.matmul(
                        p_sc[:, j, :], ident_bf[:],
                        mb_sb[:, j * 512:(j + 1) * 512],
                        start=False, stop=True, skip_group_check=True,
                    )

            # ---- softmax over n: raw exp rows (bf16) + chunked row sums;
            # no max subtraction (|score| <= ||v||_1, ~10 here) ----
            e_row = rowp.tile([BL, N], bf16, tag="e_row", name=f"e_row{t}")
            esums = rowp.tile([BL, NC4], f32, tag="esums")
            for j in range(NC4):
                js = slice(j * 512, (j + 1) * 512)
                nc.scalar.activation(e_row[:, js], p_sc[:, j, :], AF.Exp)
                nc.vector.reduce_sum(esums[:, j:j + 1], e_row[:, js],
                                     axis=AX.X)
            esum = rowp.tile([BL, 1], f32, tag="esum")
            nc.vector.reduce_sum(esum[:], esums[:], axis=AX.X)
            rinv = rowp.tile([BL, 1], f32, tag="rinv")
            nc.vector.reciprocal(rinv[:], esum[:])
            w_out = rowp.tile([BL, N], bf16, tag="w_out")
            nc.vector.tensor_scalar_mul(w_out[:], e_row[:], rinv[:])
            nc.scalar.dma_start(d_wout[:, t, :], w_out[:])
            if not last:
                wT_sb = rowp.tile([128, NC128, BL], bf16, tag="wT", bufs=1)
                for c in range(NC128):
                    p_w = pmm.tile([128, BL], bf16, tag="pr",
                                   name=f"p_w{t}_{c}")
                    nc.tensor.transpose(
                        p_w[:], w_out[:, c * 128:(c + 1) * 128],
                        ident_bf[:],
                    )
                    nc.vector.tensor_copy(wT_sb[:, c, :], p_w[:])

            if last:
                continue   # h after the final step is never observed

            # ---- vecU[b, :] = sum_n e[b, n] seq[b][n, :]; four batches
            # run concurrently in separate PE column groups ----
            p_vT = psc.tile([128, 2, BL], f32, tag="sc", name=f"p_vT{t}")
            for q in range(2):
                p_vq = pmm.tile([128, D], f32, tag="pr", name=f"p_vq{t}_{q}")
                for c in range(NC128):
                    for j in range(4):
                        b = 4 * q + j
                        nc.tensor.matmul(
                            p_vq[32 * j:32 * j + 1, :],
                            wT_sb[:, c, b:b + 1], seqO_b[b][:, c, :],
                            start=(c == 0), stop=(c == NC128 - 1),
                            tile_position=(0, 32 * j),
                            skip_group_check=True,
                        )
                for j in range(4):
                    b = 4 * q + j
                    vrow = hp.tile([1, D], f32, tag="vrow", bufs=4)
                    nc.scalar.copy(vrow[:], p_vq[32 * j:32 * j + 1, :])
                    for dh in range(2):
                        nc.tensor.transpose(
                            p_vT[:, dh, b:b + 1],
                            vrow[0:1, dh * 128:(dh + 1) * 128],
                            ident[:1, :1],
                        )
            vecT = hp.tile([128, 2, BL], bf16, tag="vecT", bufs=1)
            nc.vector.tensor_copy(vecT[:], p_vT[:])

            # ---- GRU: gates as [8(b), 768(gate)] rows; x^T/h^T [128, 8]
            # stationary, W^T [128, 768] bf16 moving. The r/z pre-acts of
            # BOTH sources accumulate into one psum region:
            #   pg[:,0,0:512]   = gx_rz + gh_rz (+ biases)
            #   pg[:,0,512:768] = gx_n + bih_n
            #   pg[:,1,512:768] = gh_n + bhh_n
            pg = psc.tile([BL, 2, 1024], f32, tag="sc", name=f"pg{t}")
            for stat, w_mov in ((vecT, wihT_sb), (hT_bf, whhT_sb)):
                for dh in range(2):
                    nc.tensor.matmul(
                        pg[:, 0, 0:512], stat[:, dh, :], w_mov[:, dh, 0:512],
                        start=(stat is vecT and dh == 0),
                        stop=(nobias and stat is not vecT and dh == 1),
                        skip_group_check=True,
                    )
            if not nobias:
                nc.tensor.matmul(pg[:, 0, 0:512], ones8[:],
                                 bias_sb[:, 0, 0:512],
                                 start=False, stop=False,
                                 skip_group_check=True)
                nc.tensor.matmul(pg[:, 0, 0:512], ones8[:],
                                 bias_sb[:, 1, 0:512],
                                 start=False, stop=True,
                                 skip_group_check=True)
            for sec, stat, w_mov in ((0, vecT, wihT_sb), (1, hT_bf, whhT_sb)):
                for dh in range(2):
                    nc.tensor.matmul(
                        pg[:, sec, 512:768], stat[:, dh, :],
                        w_mov[:, dh, 512:768],
                        start=(dh == 0), stop=(nobias and dh == 1),
                        skip_group_check=True,
                    )
                if not nobias:
                    nc.tensor.matmul(
                        pg[:, sec, 512:768], ones8[:],
                        bias_sb[:, sec, 512:768],
                        start=False, stop=True, skip_group_check=True,
                    )

            # sigmoid via 0.5*tanh(0.5x)+0.5, folded into the stt chain:
            # r,z: tau = tanh(0.5*(gx+gh));  n = tanh(xn + 0.5*(tau_r+1)*hn)
            # h' = n + 0.5*(tau_z+1)*(h-n)
            tau = hp.tile([BL, 512], f32, tag="tau", bufs=1)
            nc.scalar.activation(tau[:], pg[:, 0, 0:512], AF.Tanh, scale=0.5)
            t1s = hp.tile([BL, H], f32, tag="t1s", bufs=1)
            nc.vector.scalar_tensor_tensor(
                t1s[:], tau[:, 0:256], 1.0, pg[:, 1, 512:768],
                op0=ALU.add, op1=ALU.mult,
            )
            n_in = hp.tile([BL, H], f32, tag="n_in", bufs=1)
            nc.vector.scalar_tensor_tensor(
                n_in[:], t1s[:], 0.5, pg[:, 0, 512:768],
                op0=ALU.mult, op1=ALU.add,
            )
            n_t = hp.tile([BL, H], f32, tag="n_t", bufs=1)
            nc.scalar.activation(n_t[:], n_in[:], AF.Tanh)
            d_hn = hp.tile([BL, H], f32, tag="d_hn", bufs=1)
            nc.vector.tensor_tensor(d_hn[:], h_rows[:], n_t[:],
                                    op=ALU.subtract)
            e2 = hp.tile([BL, H], f32, tag="e2", bufs=1)
            nc.vector.scalar_tensor_tensor(
                e2[:], tau[:, 256:512], 1.0, d_hn[:],
                op0=ALU.add, op1=ALU.mult,
            )
            h_new = hp.tile([BL, H], f32, tag="h_rows")
            nc.vector.scalar_tensor_tensor(
                h_new[:], e2[:], 0.5, n_t[:], op0=ALU.mult, op1=ALU.add,
            )
            h_rows = h_new
            h_bf = hp.tile([BL, H], bf16, tag="h_bf", bufs=1)
            nc.vector.tensor_copy(h_bf[:], h_new[:])
            p_hT = pmm.tile([128, 2, BL], bf16, tag="pr", name=f"p_hT{t}")
            for dh in range(2):
                nc.tensor.transpose(
                    p_hT[:, dh, :], h_bf[:, dh * 128:(dh + 1) * 128],
                    ident_bf[:],
                )
            hT_new = hp.tile([128, 2, BL], bf16, tag="hT")
            nc.vector.tensor_copy(hT_new[:], p_hT[:])
            hT_bf = hT_new

    if split:
        _split_multiwaits(nc, mybir)
    return nc


def _split_multiwaits(nc, mybir):
    """Walrus gives each lowered TPB instruction a single sem-wait slot;
    Tile happily emits several. Peel surplus waits onto same-engine NoOps
    inserted right before the instruction (semantically identical: the
    engine stalls at the same program point)."""
    skip = ("InstNoOp", "InstEventSemaphore")
    for f in nc.m.functions:
        for blk in f.blocks:
            out, changed = [], False
            for ins in blk.instructions:
                si = ins.sync_info
                if (si is not None and len(si.on_wait) > 1
                        and type(ins).__name__ not in skip):
                    waits = list(si.on_wait)
                    for i, w in enumerate(waits[:-1]):
                        out.append(mybir.InstNoOp(
                            name=f"{ins.name}-w{i}",
                            engine=ins.engine,
                            sync_info=mybir.SyncInfo(on_wait=[w], on_update=[]),
                            bass_nofuse=True,
                        ))
                    ins.sync_info = mybir.SyncInfo(
                        on_wait=[waits[-1]], on_update=list(si.on_update))
                    changed = True
                out.append(ins)
            if changed:
                blk.instructions = out


def _get_program(T: int, nomask: bool, nobias: bool):
    key = (T, nomask, nobias)
    if key not in _CACHE:
        _CACHE[key] = _build(T, nomask, nobias)
    return _CACHE[key]


def _prep_core(seq_c, hid_c, mask_c, W_seq, W_h, v_att, W_ih, W_hh, b_ih, b_hh,
               nomask, nobias):
    """Host-side layout prep for one core's batch slice."""
    bf16 = ml_dtypes.bfloat16
    f32 = np.float32
    # seqT [128, BL, 2, N]: seqT[r, b, dh, n] = seq[b, n, dh*128+r]
    seqT = np.ascontiguousarray(
        seq_c.transpose(2, 0, 1).reshape(2, 128, BL, N).transpose(1, 2, 0, 3)
    ).astype(bf16)
    # seqO [128, BL, N/128, D]: seqO[r, b, c, d] = seq[b, c*128+r, d]
    seqO = np.ascontiguousarray(
        seq_c.reshape(BL, N // 128, 128, D).transpose(2, 0, 1, 3)
    ).astype(bf16)

    wpack = np.zeros((128, 2, _WPACK), dtype=f32)
    wpack[:, :, _OFF_WH:_OFF_WH + P] = W_h.reshape(2, 128, P).transpose(1, 0, 2)
    wpack[:, :, _OFF_WSEQ:_OFF_WSEQ + P] = \
        W_seq.reshape(2, 128, P).transpose(1, 0, 2)
    wpack[:, :, _OFF_WIH:_OFF_WIH + 3 * H] = \
        W_ih.transpose(1, 0).reshape(2, 128, 3 * H).transpose(1, 0, 2)
    wpack[:, :, _OFF_WHH:_OFF_WHH + 3 * H] = \
        W_hh.transpose(1, 0).reshape(2, 128, 3 * H).transpose(1, 0, 2)
    vr = v_att.reshape(2, 128).transpose(1, 0)  # [128, 2]
    for b in range(BL):
        wpack[:, :, _OFF_V8 + 8 * b + b] = vr
    wpack[:, :, _OFF_H0T:_OFF_H0T + BL] = \
        hid_c.transpose(1, 0).reshape(2, 128, BL).transpose(1, 0, 2)
    im = {
        "seqT": seqT, "seqO": seqO,
        "wpack": wpack.astype(bf16),
        "h0r": np.ascontiguousarray(hid_c).astype(f32),
    }
    if not nobias:
        im["bias"] = np.stack([b_ih, b_hh]).reshape(1, 2, 3 * H).astype(bf16)
    if not nomask:
        im["mb"] = np.where(mask_c > 0, 0.0, NEG_INF).astype(f32)
    return im


def kernel(sequence, hidden_t, sequence_mask, num_steps,
           W_seq, W_h, v_att, W_ih, W_hh, b_ih, b_hh):
    from concourse.bass_utils import run_bass_kernel_spmd

    T = int(num_steps)
    sequence = np.asarray(sequence, np.float32)
    hidden_t = np.asarray(hidden_t, np.float32)
    sequence_mask = np.asarray(sequence_mask, np.float32)
    W_seq = np.asarray(W_seq, np.float32)
    W_h = np.asarray(W_h, np.float32)
    v_att = np.asarray(v_att, np.float32)
    W_ih = np.asarray(W_ih, np.float32)
    W_hh = np.asarray(W_hh, np.float32)
    b_ih = np.asarray(b_ih, np.float32)
    b_hh = np.asarray(b_hh, np.float32)

    nomask = bool(np.all(sequence_mask > 0))
    nobias = bool(np.all(b_ih == 0) and np.all(b_hh == 0))
    nc = _get_program(T, nomask, nobias)
    in_maps = []
    for c in range(NCORES):
        sl = slice(c * BL, (c + 1) * BL)
        in_maps.append(_prep_core(
            sequence[sl], hidden_t[sl], sequence_mask[sl],
            W_seq, W_h, v_att, W_ih, W_hh, b_ih, b_hh, nomask, nobias,
        ))
    kr = run_bass_kernel_spmd(
        nc, in_maps, list(range(NCORES)), **_RUN_KWARGS,
    )
    globals()["_LAST_RESULTS"] = kr
    res = kr.results
    out = np.concatenate([res[c]["wout"] for c in range(NCORES)], axis=0)
    return out.astype(np.float32)


# test-harness hooks (unused in grading): set _RUN_KWARGS = {"trace": True}
# before calling kernel() to get NTFF profile info in _LAST_RESULTS.
_RUN_KWARGS = {}
_LAST_RESULTS = None
